# revision 18
# baseline (speedup 1.0000x reference)
"""Trainium2 Bass kernel: DagnabbitAutoEncoder sequential DAG sweep.

Strategy (8 NeuronCores, SPMD single program, per-core data):
  - Host computes topological levels and a deadline-forced pass schedule:
    nodes are batched into per-(stage, local-type-slot) passes; 28 stages
    (= DAG depth), one AllGather per stage to exchange computed
    embeddings (fp16) through shared DRAM buffers.
  - Trunk encoder types are partitioned 4-per-core; the shared output-node
    encoder is replicated (local slot 4). ALL five per-core weight blobs
    (W1 + repacked W2, fp16) stay RESIDENT in SBUF (5 x 12KB/partition),
    loaded once at program start: a pass's weights are addressed
    statically by its local-type-index, so there is zero per-pass weight
    traffic. Slots are keyed by local type index so the one instruction
    stream works for all cores; per-core G imbalance inside a slot is
    topped up with that core's own eligible future nodes of the same
    type (productive padding).
  - Parent embeddings are fetched with dma_gather(transpose=True), which
    lands gathered rows as columns: exactly the X^T [512, G] layout the
    tensor engine needs.
  - Stage 1 GEMM: W1 tiles stationary, X^T moving -> H^T in PSUM; ScalarE
    applies bias+exact-GELU (the only scalar table used -> no
    ACT_TABLE_LOAD thrash) and casts to fp16.
    Stage 2 GEMM: H^T stationary, W2 tiles moving -> Y [g,256] in PSUM;
    bias2 via a ones-row K=1 matmul. Row normalization to sqrt(D) runs
    entirely on the (otherwise idle) Vector engine: squared-row-sum via
    tensor_tensor_reduce, then a Quake-style bitcast rsqrt with two
    Newton iterations.
  - Results are stored fp16 once per stage (ccin -> oout + AllGather).
"""

import math
import sys

import numpy as np

if "/opt/trn_rl_repo" not in sys.path:
    sys.path.insert(0, "/opt/trn_rl_repo")

NCORES = 8
GCAP = 256  # max nodes per pass slot
TYPES_PER_CORE = 4
QUAKE_C = 0x5F3759DF


# --------------------------------------------------------------------------
# Host-side schedule
# --------------------------------------------------------------------------

class Plan:
    pass


def _build_plan(node_inputs, node_types, num_roots, num_trunk, num_out):
    N = node_inputs.shape[0]
    out_start = num_trunk + num_roots
    is_out = node_types >= out_start
    enc = np.where(is_out, num_trunk, np.clip(node_types, 0, num_trunk - 1))

    # ASAP levels
    level = np.zeros(N, np.int64)
    ni = node_inputs
    for n in range(num_roots, N):
        i0, i1 = ni[n]
        level[n] = (level[i0] + 1) if is_out[n] else max(level[i0], level[i1]) + 1
    S = int(level.max())

    # ALAP deadlines
    alap = np.full(N, S, np.int64)
    for n in range(N - 1, num_roots - 1, -1):
        i0, i1 = ni[n]
        a = alap[n] - 1
        if alap[i0] > a:
            alap[i0] = a
        if (not is_out[n]) and alap[i1] > a:
            alap[i1] = a

    # --- optimize the trunk-type -> core pinning ---------------------------
    # Forced-only pre-pass to get per-stage forced type sets, then greedily
    # partition types 4-per-core to minimize per-stage forced imbalance.
    def _forced_sets():
        sched = np.zeros(N, bool)
        sched[:num_roots] = True
        rem = list(range(num_roots, N))
        sets = []
        while rem:
            elig = [n for n in rem
                    if sched[ni[n][0]] and (is_out[n] or sched[ni[n][1]])]
            by = {}
            for n in elig:
                by.setdefault(int(enc[n]), []).append(n)
            forced = {t for t, nodes in by.items()
                      if any(alap[n] == len(sets) + 1 for n in nodes)}
            now = []
            for t in forced:
                now.extend(by[t])
            sets.append({t for t in forced if t < num_trunk})
            for n in now:
                sched[n] = True
            nowset = set(now)
            rem = [n for n in rem if n not in nowset]
        return sets

    fsets = _forced_sets()
    presence = {t: [s for s, fs in enumerate(fsets) if t in fs]
                for t in range(num_trunk)}
    order = sorted(range(num_trunk), key=lambda t: -len(presence[t]))
    cap = [TYPES_PER_CORE] * NCORES
    cnt = [[0] * NCORES for _ in fsets]
    t2c = {}
    for t in order:
        best, bestcost = None, None
        for c in range(NCORES):
            if cap[c] == 0:
                continue
            cost = sum(1 for s in presence[t]
                       if cnt[s][c] + 1 > max(cnt[s]))
            if bestcost is None or cost < bestcost:
                best, bestcost = c, cost
        t2c[t] = best
        cap[best] -= 1
        for s in presence[t]:
            cnt[s][best] += 1
    core_types = [[] for _ in range(NCORES)]
    for t in range(num_trunk):
        core_types[t2c[t]].append(t)
    lti_map = {}
    for c in range(NCORES):
        for j, t in enumerate(core_types[c]):
            lti_map[t] = j
    # type at (core, lti)
    type_at = [[core_types[c][j] for j in range(TYPES_PER_CORE)]
               for c in range(NCORES)]

    # --- main greedy: deadline-forced scheduling, lti-keyed slots ---------
    scheduled = np.zeros(N, bool)
    scheduled[:num_roots] = True
    remaining = list(range(num_roots, N))

    stages = []
    for s in range(1, S + 1):
        elig = [
            n
            for n in remaining
            if scheduled[ni[n][0]] and (is_out[n] or scheduled[ni[n][1]])
        ]
        by_enc = {}
        for n in elig:
            by_enc.setdefault(int(enc[n]), []).append(n)
        forced = {
            t for t, nodes in by_enc.items() if any(alap[n] == s for n in nodes)
        }
        if not forced:
            continue

        slots = []  # (lti, percore list of node lists, G)

        # trunk types, grouped by local type index
        by_lti = {}
        for t in sorted(forced):
            if t == num_trunk:
                continue
            by_lti.setdefault(lti_map[t], {})[t2c[t]] = by_enc.pop(t)
        for lti in sorted(by_lti):
            coremap = by_lti[lti]
            mult = max((len(v) + GCAP - 1) // GCAP for v in coremap.values())
            for j in range(mult):
                percore = [list(coremap.get(c, [])[j * GCAP:(j + 1) * GCAP])
                           for c in range(NCORES)]
                G = max(len(p) for p in percore)
                # top-up: cores short of G pull their own eligible nodes of
                # their type at this lti (most-urgent first)
                if j == mult - 1:
                    for c in range(NCORES):
                        need = G - len(percore[c])
                        if need <= 0:
                            continue
                        tc = type_at[c][lti]
                        pool = by_enc.get(tc)
                        if not pool:
                            continue
                        pool.sort(key=lambda n: alap[n])
                        take = pool[:need]
                        by_enc[tc] = pool[need:]
                        if not by_enc[tc]:
                            del by_enc[tc]
                        percore[c].extend(take)
                slots.append((lti, percore, G))

        # output nodes: replicated encoder, split evenly across cores
        if num_trunk in forced:
            pool = by_enc.pop(num_trunk)
            Gout = (len(pool) + NCORES - 1) // NCORES
            for a in range(0, len(pool), Gout * NCORES):
                chunk = pool[a : a + Gout * NCORES]
                g = (len(chunk) + NCORES - 1) // NCORES
                percore = [chunk[c * g:(c + 1) * g] for c in range(NCORES)]
                slots.append((TYPES_PER_CORE, percore, g))

        # biggest slots first
        slots.sort(key=lambda sl: -sl[2])

        newly = []
        for _, percore, _ in slots:
            for lst in percore:
                newly.extend(lst)
        for n in newly:
            scheduled[n] = True
        newset = set(newly)
        remaining = [n for n in remaining if n not in newset]

        stages.append(dict(s=s, slots=slots, Gs=[sl[2] for sl in slots]))

    assert not remaining, f"{len(remaining)} nodes unscheduled"

    # ---- layout: bufH rows, cc offsets, output rows, global slot ids ----
    plan = Plan()
    plan.S = len(stages)
    plan.stages = stages
    slot_id = 0
    bufh_off = 0  # offset after the 128 static rows
    out_off = 0
    stage_of_node = np.full(N, -1, np.int64)
    within_of_node = np.full(N, -1, np.int64)
    outpos_of_node = {}
    for sidx, st in enumerate(stages):
        Gs = st["Gs"]
        K = len(Gs)
        st["slot_ids"] = list(range(slot_id, slot_id + K))
        slot_id += K
        R = sum(Gs)
        st["R"] = R
        st["off"] = bufh_off
        st["out_off"] = out_off
        pre = np.concatenate([[0], np.cumsum(Gs)]).astype(int)
        st["pre"] = pre
        # per-slot gather sizes (rows; dma_gather transpose mode requires
        # 128-alignment) so each slot's gather can issue independently
        st["NXs"] = [((2 * G + 127) // 128) * 128 for G in Gs]
        st["NX"] = sum(st["NXs"])
        for c in range(NCORES):
            for k, (lti, percore, G) in enumerate(st["slots"]):
                for i, n in enumerate(percore[c]):
                    stage_of_node[n] = sidx
                    within_of_node[n] = c * R + pre[k] + i
                    outpos_of_node[n] = (c, out_off + pre[k] + i)
        bufh_off += NCORES * R
        out_off += R
    plan.bufH_rows = 128 + bufh_off
    plan.R_tot = out_off
    plan.slots_tot = slot_id
    plan.Rmax = max(st["R"] for st in stages)
    plan.NXmax = max(st["NX"] for st in stages)
    plan.Gmax = max(max(st["Gs"]) for st in stages)
    assert plan.bufH_rows < 32768, plan.bufH_rows
    plan.stage_rowoff = [128 + st["off"] for st in stages]
    plan.stage_of_node = stage_of_node
    plan.within_of_node = within_of_node
    plan.outpos_of_node = outpos_of_node
    plan.enc = enc
    plan.is_out = is_out
    plan.N = N
    plan.num_roots = num_roots
    plan.num_trunk = num_trunk
    plan.num_out = num_out
    plan.out_start = out_start
    plan.node_inputs = node_inputs
    plan.node_types = node_types
    plan.core_types = core_types
    return plan


def _wrap_idxs(idx_list, num_idxs):
    """int16 index layout for dma_gather: [128, num_idxs//16], index i at
    partition i%16, column i//16, replicated across the 8 Q7 16-partition
    groups."""
    a = np.zeros(num_idxs, np.int16)
    a[: len(idx_list)] = np.asarray(idx_list, np.int16)
    a = a.reshape(num_idxs // 16, 16).T  # [16, cols]
    return np.tile(a, (8, 1))  # [128, cols]


def _build_core_inputs(plan, core, W1, b1, W2, b2, root_emb, output_slot_emb):
    """Per-core input arrays (shapes identical across cores)."""
    num_trunk = plan.num_trunk
    D = root_emb.shape[1]
    H = W1.shape[2]
    assert D == 256 and H == 1024 and W1.shape[1] == 2 * D
    ni_types = list(plan.core_types[core]) + [num_trunk]

    # weight blob: per local type, 768 rows of 1024 fp16
    # rows 0..511   = W1[t]  (512 x 1024)
    # rows 512..767 = repacked W2[t]: blob[512 + q*128 + p] =
    #                 concat_j W2[t][(4q+j)*128 + p, :]  (j = 0..3)
    rows_per = 3 * D  # 768
    blob = np.zeros((5 * rows_per, H), np.float16)
    for li, t in enumerate(ni_types):
        w1 = W1[t].astype(np.float16)  # [512, 1024]
        blob[li * rows_per : li * rows_per + 2 * D] = w1
        w2 = W2[t].astype(np.float16).reshape(2, 4, 128, D)
        w2 = w2.transpose(0, 2, 1, 3).reshape(2 * 128, 4 * D)  # [256, 1024]
        blob[li * rows_per + 2 * D : (li + 1) * rows_per] = w2
    # permute for direct [128, 30, 1024] SBUF residency:
    # dram row p*30 + j  =  blob row j*128 + p
    nblk = 5 * rows_per // 128  # 30
    wres = blob.reshape(nblk, 128, H).transpose(1, 0, 2).reshape(128, nblk * H)

    # per-slot tables
    xidx_cols = []
    bias1 = np.zeros((128, plan.slots_tot * 8), np.float32)
    bias2 = np.zeros((1, plan.slots_tot * D), np.float16)
    nH = H // 128  # number of 128-row b1 tiles (8)
    for st in plan.stages:
        for k, (lti, percore, G) in enumerate(st["slots"]):
            sl = st["slot_ids"][k]
            t = ni_types[lti]
            bias1[:, sl * nH : (sl + 1) * nH] = (
                b1[t].astype(np.float32).reshape(nH, 128).T
            )
            bias2[0, sl * D : (sl + 1) * D] = b2[t].astype(np.float16)
            nodes = percore[core]
            e0 = []
            e1 = []
            for n in nodes:
                i0, i1 = plan.node_inputs[n]
                e0.append(_node_row(plan, i0))
                if plan.is_out[n]:
                    e1.append(64 + int(plan.node_types[n]) - plan.out_start)
                else:
                    e1.append(_node_row(plan, i1))
            e0 += [0] * (G - len(nodes))
            e1 += [0] * (G - len(nodes))
            xidx_cols.append(_wrap_idxs(e0 + e1, st["NXs"][k]))

    xidx = np.concatenate(xidx_cols, axis=1)

    initr = np.zeros((128, D), np.float16)
    initr[: plan.num_roots] = root_emb.astype(np.float16)
    initr[64 : 64 + plan.num_out] = output_slot_emb.astype(np.float16)

    return dict(wblob=wres, xidx=xidx, bias1=bias1, bias2=bias2,
                initr=initr)


def _node_row(plan, n):
    n = int(n)
    if n < plan.num_roots:
        return n
    s = int(plan.stage_of_node[n])
    assert s >= 0, n
    return plan.stage_rowoff[s] + int(plan.within_of_node[n])


# --------------------------------------------------------------------------
# Bass program
# --------------------------------------------------------------------------

def _build_nc(plan, D, H, gelu_mode="act", repeat=1, xmode="shared",
              skip_ag=False, norm_mode="batched", cc_on_sync=False):
    import concourse.bacc as bacc
    import concourse.mybir as mybir
    from concourse import tile
    from concourse.bass import _add_dep_helper

    dt = mybir.dt
    AF = mybir.ActivationFunctionType
    ALU = mybir.AluOpType
    rows_per = 3 * D  # 768
    nblk = 5 * rows_per // 128  # 30

    nc = bacc.Bacc("TRN2", target_bir_lowering=False, debug=False,
                   enable_asserts=False, num_devices=NCORES)

    wblob = nc.dram_tensor("wblob", [128, nblk * H], dt.float16,
                           kind="ExternalInput")
    xidx_cols = sum(st["NX"] for st in plan.stages) // 16
    xidx = nc.dram_tensor("xidx", [128, xidx_cols], dt.int16,
                          kind="ExternalInput")
    bias1 = nc.dram_tensor("bias1", [128, plan.slots_tot * 8], dt.float32,
                           kind="ExternalInput")
    bias2 = nc.dram_tensor("bias2", [1, plan.slots_tot * D], dt.float16,
                           kind="ExternalInput")
    initr = nc.dram_tensor("initr", [128, D], dt.float16, kind="ExternalInput")
    oout = nc.dram_tensor("oout", [plan.R_tot, D], dt.float16,
                          kind="ExternalOutput")

    RG = [list(range(NCORES))]

    with tile.TileContext(nc) as tc:
        with (
            tc.tile_pool(name="dram", bufs=1, space="DRAM") as dpool,
            tc.tile_pool(name="ccpool", bufs=2, space="DRAM") as ccpool,
            tc.tile_pool(name="cpool", bufs=1) as cpool,
            tc.tile_pool(name="xpool", bufs=2) as xpool,
            tc.tile_pool(name="hpool", bufs=2) as hpool,
            tc.tile_pool(name="ypool", bufs=3) as ypool,
            tc.tile_pool(name="phpool", bufs=1, space="PSUM") as phpool,
            tc.tile_pool(name="pypool", bufs=3, space="PSUM") as pypool,
        ):
            statics = dpool.tile([128, D], dt.float16,
                                 addr_space="Shared", name="statics")
            gbase = statics
            ago_tiles = []  # rep-0 per-stage AG output tiles

            # resident weights: 5 blobs x [128, 6, 1024] fp16
            wsb = cpool.tile([128, nblk, H], dt.float16, name="wsb")
            nc.sync.dma_start(wsb[:, :, :], wblob.ap())

            xidx_sb = cpool.tile(list(xidx.shape), dt.int16, name="xidx_sb")
            nc.sync.dma_start(xidx_sb[:, :], xidx.ap())
            bias1_sb = cpool.tile(list(bias1.shape), dt.float32, name="bias1_sb")
            nc.sync.dma_start(bias1_sb[:, :], bias1.ap())
            bias2_sb = cpool.tile(list(bias2.shape), dt.float16, name="bias2_sb")
            nc.sync.dma_start(bias2_sb[:, :], bias2.ap())

            init_sb = cpool.tile([128, D], dt.float16, name="init_sb")
            nc.sync.dma_start(init_sb[:, :], initr.ap())
            nc.sync.dma_start(gbase[0:128, :], init_sb[:, :])

            ones_sb = cpool.tile([1, 128], dt.float16, name="ones_sb")
            nc.gpsimd.memset(ones_sb[:, :], 1.0)
            eps_sb = cpool.tile([128, 1], dt.float32, name="eps_sb")
            nc.gpsimd.memset(eps_sb[:, :], 1e-24)
            one32_sb = cpool.tile([128, 1], dt.float32, name="one32_sb")
            nc.gpsimd.memset(one32_sb[:, :], 1.0)

            prev_cc = None
            for rep in range(repeat):
              xoff = 0
              for si0, st in enumerate(plan.stages):
                si = f"{rep}_{si0}"
                R, Gs = st["R"], st["Gs"]
                pre = st["pre"]
                # one gather per slot: slot 0's (small) gather completes
                # quickly after the AG; later slots' descriptor generation
                # overlaps with slot 0's compute
                xts = []
                for k, NXk in enumerate(st["NXs"]):
                    xtk = xpool.tile([128, 2, NXk], dt.float16, tag="xt",
                                     bufs=4, name=f"xt{si}_{k}")
                    g_inst = nc.gpsimd.dma_gather(
                        xtk[:, :, :], gbase[:, :],
                        xidx_sb[:, xoff : xoff + NXk // 16],
                        NXk, NXk, D, transpose=True,
                    )
                    if prev_cc is not None:
                        _add_dep_helper(g_inst.ins, prev_cc.ins, True,
                                        "gather reads prior AG outputs")
                    xoff += NXk // 16
                    xts.append(xtk)

                ccin = ccpool.tile([R, D], dt.float16, tag=f"cc{si}",
                                   bufs=1, name=f"cc{si}")

                nch = sum((G + 127) // 128 for G in Gs)
                ssall = ypool.tile([128, nch], dt.float32, tag="ssall",
                                   name=f"ssall{si}")
                invall = ypool.tile([128, nch], dt.float32, tag="invall",
                                    name=f"invall{si}")
                chunks = []  # (y16 tile, g, pre+c0, chunk idx)
                jch = 0
                for k, (lti, _percore, G) in enumerate(st["slots"]):
                    sl = st["slot_ids"][k]
                    xtk = xts[k]
                    Gp = 64
                    while Gp < G:
                        Gp *= 2
                    wb = lti * 6  # this slot's resident blob base

                    ph = phpool.tile([128, 8 * Gp], dt.float32, tag="ph",
                                     name=f"ph_{si}_{k}")
                    for m in range(8):
                        for kk in range(4):
                            rhs = xtk[:, kk % 2,
                                      (kk // 2) * G : (kk // 2) * G + G]
                            nc.tensor.matmul(
                                ph[:, m * Gp : m * Gp + G],
                                wsb[:, wb + kk, m * 128 : (m + 1) * 128],
                                rhs,
                                start=(kk == 0), stop=(kk == 3),
                            )
                    hsb = hpool.tile([128, 8, Gp], dt.float16, tag="h",
                                     name=f"h_{si}_{k}")
                    for m in range(8):
                        pslice = ph[:, m * Gp : m * Gp + G]
                        bslice = bias1_sb[:, sl * 8 + m : sl * 8 + m + 1]
                        nc.scalar.activation(hsb[:, m, 0:G], pslice,
                                             AF.Gelu, bias=bslice)
                    for c0 in range(0, G, 128):
                        g = min(128, G - c0)
                        py = pypool.tile([128, D], dt.float32, tag="py",
                                         name=f"py_{si}_{k}_{c0}")
                        for kk in range(8):
                            nc.tensor.matmul(
                                py[0:g, :],
                                hsb[:, kk, c0 : c0 + g],
                                wsb[:, wb + 4 + kk // 4,
                                    (kk % 4) * D : (kk % 4 + 1) * D],
                                start=(kk == 0), stop=False,
                            )
                        nc.tensor.matmul(
                            py[0:g, :], ones_sb[0:1, 0:g],
                            bias2_sb[0:1, sl * D : (sl + 1) * D],
                            start=False, stop=True,
                        )
                        # squared-row-sum (Square is in every act table ->
                        # no table load); y16 = unscaled fp16 copy of py
                        sq = ypool.tile([128, D], dt.float16, tag="sq",
                                        name=f"sq_{si}_{k}_{c0}")
                        nc.scalar.activation(
                            sq[0:g, :], py[0:g, :], AF.Square,
                            accum_out=ssall[0:g, jch : jch + 1])
                        y16 = ypool.tile([128, D], dt.float16, tag="y16",
                                         bufs=8, name=f"y16_{si}_{k}_{c0}")
                        nc.vector.tensor_scalar_mul(y16[0:g, :], py[0:g, :],
                                                    one32_sb[0:g, :])
                        chunks.append((y16, g, pre[k] + c0, jch))
                        jch += 1

                # one batched rsqrt for the whole stage: a single table
                # switch pair per stage instead of one per chunk
                nc.scalar.activation(invall[:, :], ssall[:, :],
                                     AF.Abs_reciprocal_sqrt,
                                     scale=1.0 / D, bias=eps_sb[:, :])
                for y16, g, row, j in chunks:
                    y16s = ypool.tile([128, D], dt.float16, tag="y16s",
                                      bufs=8, name=f"y16s_{si}_{row}")
                    nc.vector.tensor_scalar_mul(y16s[0:g, :], y16[0:g, :],
                                                invall[0:g, j : j + 1])
                    nc.sync.dma_start(ccin[row : row + g, :], y16s[0:g, :])

                # final output rows for this stage (fp16)
                r0 = st["out_off"]
                nc.sync.dma_start(oout.ap()[r0 : r0 + R, :], ccin[0:R, :])

                if si0 == len(plan.stages) - 1:
                    if rep == 0:
                        ago_tiles.append(None)
                    continue
                ago = dpool.tile([NCORES * R, D], dt.float16,
                                 addr_space="Shared", bufs=1,
                                 name=f"ago{si}")
                if rep == 0:
                    ago_tiles.append(ago)
                ag_out = ago[:, :]
                cc_eng = nc.sync if cc_on_sync else nc.gpsimd
                prev_cc = cc_eng.collective_compute(
                    "AllGather",
                    mybir.AluOpType.bypass,
                    replica_groups=RG,
                    ins=[ccin[0:R, :]],
                    outs=[ag_out],
                    unique_tensors="Yes",
                )

    nc.compile()

    base_addr = nc.lookup_mls(gbase.tensor).memorylocations[0].addr
    rowbytes = D * 2
    rowoff = []
    for sidx, ago in enumerate(ago_tiles):
        if ago is None:
            rowoff.append(0)  # last stage: never referenced by gathers
            continue
        a = nc.lookup_mls(ago.tensor).memorylocations[0].addr
        off = a - base_addr
        assert off % rowbytes == 0, (sidx, off)
        r = off // rowbytes
        assert 0 < r and r + ago.shape[0] < 32768, (sidx, r)
        rowoff.append(int(r))
    plan.stage_rowoff = rowoff
    return nc


# --------------------------------------------------------------------------
# Entry point
# --------------------------------------------------------------------------

_CACHE = {}


def _prepare(node_inputs, node_types, num_roots, num_trunk, num_out):
    key = (node_inputs.tobytes(), node_types.tobytes())
    if key in _CACHE:
        return _CACHE[key]
    plan = _build_plan(node_inputs, node_types, num_roots, num_trunk, num_out)
    _CACHE[key] = plan
    return plan


def kernel(node_inputs_indices, node_types, root_emb, output_slot_emb,
           W1, b1, W2, b2):
    node_inputs_indices = np.asarray(node_inputs_indices)
    node_types = np.asarray(node_types)
    root_emb = np.asarray(root_emb, np.float32)
    output_slot_emb = np.asarray(output_slot_emb, np.float32)
    W1 = np.asarray(W1, np.float32)
    b1 = np.asarray(b1, np.float32)
    W2 = np.asarray(W2, np.float32)
    b2 = np.asarray(b2, np.float32)

    num_trunk = W1.shape[0] - 1
    num_roots = root_emb.shape[0]
    num_out = output_slot_emb.shape[0]
    plan = _prepare(node_inputs_indices, node_types, num_roots, num_trunk,
                    num_out)
    D = root_emb.shape[1]
    H = W1.shape[2]

    nckey = ("nc", node_inputs_indices.tobytes(), node_types.tobytes())
    if nckey in _CACHE:
        nc = _CACHE[nckey]
    else:
        nc = _build_nc(plan, D, H)
        _CACHE[nckey] = nc

    in_maps = [
        _build_core_inputs(plan, c, W1, b1, W2, b2, root_emb, output_slot_emb)
        for c in range(NCORES)
    ]

    import os

    from concourse import bass_utils
    trace = bool(int(os.environ.get("DAG_KERNEL_TRACE", "0")))
    try:
        res = bass_utils.run_bass_kernel_spmd(nc, in_maps, list(range(NCORES)),
                                              trace=trace)
    except ModuleNotFoundError:
        res = bass_utils.run_bass_kernel_spmd(nc, in_maps, list(range(NCORES)),
                                              trace=False)
    global LAST_RESULTS
    LAST_RESULTS = res
    outs = [res.results[c]["oout"] for c in range(NCORES)]

    full = np.zeros((plan.N, D), np.float32)
    full[: plan.num_roots] = root_emb
    for n, (c, r) in plan.outpos_of_node.items():
        full[n] = outs[c][r]
    return full
